# revision 1
# baseline (speedup 1.0000x reference)
"""Trainium2 Bass kernel for nn_EquivariantLayer (spectral equivariant layer).

Strategy (data-parallel over batch, 2 samples/core x 8 cores):
  All FFTs are expressed as real DFT matmuls on the TensorEngine with layouts
  chosen so no corner-turn transposes are ever needed:

    stage1:  A = f^T @ [ExR^T | ExI^T]          (contract x; out [y, (RI,kx)])
    stage2:  F = Ey @ A                          (contract y; out [c, kx], c-major)
             -> two layouts: conv layout [(i%4)*32+c, kx] and fr layout [c, (i,kx)]
    conv:    M = F (*) K elementwise (K = rfft2(sym kernel) is REAL since the
             symmetrized kernel is D4-symmetric); i-reduction via a selector
             matmul on the TensorEngine (PSUM accumulation over i-halves)
    uncurl:  TO_U = i*t, TO_V = i*s are pure-imaginary -> 2 real mults each
    synth:   field = Re(P @ B^T_cm @ Q^T) as two matmul stages (stage a/b)
    cross:   u_a v_b - u_b v_a on the VectorEngine with zero-step broadcast APs

Output [16, 128, 128, 128] f32 (~134 MB) dominates traffic (memory regime).
"""
import sys
import numpy as np

if '/opt/trn_rl_repo' not in sys.path:
    sys.path.insert(0, '/opt/trn_rl_repo')

import concourse.bass as bass
from concourse import bacc
import concourse.mybir as mybir
import concourse.tile as tile
from concourse.bass import AP
from concourse.bass_utils import run_bass_kernel_spmd

F32 = mybir.dt.float32
N_CORES = 8
B_PER_CORE = 2
C1, C2, N1, N2 = 8, 16, 64, 128
NCH_OUT = 128  # 8 fr + 120 cross

I_IDX, J_IDX = np.triu_indices(C2, 1)
_PAIR_IDX = {}
for _p, (_a, _b) in enumerate(zip(I_IDX, J_IDX)):
    _PAIR_IDX[(int(_a), int(_b))] = _p


# ---------------------------------------------------------------------------
# host-side constant construction
# ---------------------------------------------------------------------------

def _host_consts():
    x = np.arange(64)
    kx = np.arange(64)
    c = np.arange(32)
    y = np.arange(64)
    X = np.arange(128)
    Y = np.arange(128)

    FRs = np.where(kx <= 32, kx, kx - 64).astype(np.float64)  # signed row freq

    ExR = np.cos(2 * np.pi * np.outer(kx, x) / 64)   # [kx, x]
    ExI = -np.sin(2 * np.pi * np.outer(kx, x) / 64)
    # [A_R | A_I | -A_R] so stage2 fuses R/I into two matmuls
    ExF = np.concatenate([ExR.T, ExI.T, -ExR.T], axis=1)   # [x, 192]

    # F_R = C A_R + S A_I ; F_I = C A_I + S (-A_R)   (C=cos, S=sin)
    EyCT = np.cos(2 * np.pi * np.outer(c, y) / 64).T   # [y=64, c=32]
    EyST = np.sin(2 * np.pi * np.outer(c, y) / 64).T

    S_sel = np.zeros((128, 32))
    for im in range(4):
        S_sel[im * 32 + np.arange(32), np.arange(32)] = 1.0

    den = FRs[None, :] ** 2 + c[:, None].astype(np.float64) ** 2
    den[0, 0] = 1.0
    t_u = c[:, None] / den                           # [32, 64]
    s_v = -FRs[None, :] / den
    t_rep = np.tile(t_u, (1, 8))                     # [32, 512] (j-rep)
    s_rep = np.tile(s_v, (1, 8))
    tsg = np.concatenate([-t_rep, t_rep, -s_rep, s_rep], axis=1)  # [32, 2048]

    w_c = np.where(c == 0, 1.0, 2.0)
    s_q = 2.0 / (128.0 * 128.0)
    QRT = (s_q * w_c[None, :] * np.cos(2 * np.pi * np.outer(Y, c) / 128)).T  # [c, Y]
    QIT = (s_q * w_c[None, :] * np.sin(2 * np.pi * np.outer(Y, c) / 128)).T
    QF1 = np.concatenate([QRT, QIT], axis=1)         # [32, 256]
    QF2 = np.concatenate([-QIT, QRT], axis=1)

    PRT = np.cos(2 * np.pi * np.outer(FRs, X) / 128)   # [r=64, X=128]
    PIT = np.sin(2 * np.pi * np.outer(FRs, X) / 128)
    PRT[32, :] = 0.0
    PIT[32, :] = 0.0
    PRTPnIT = np.concatenate([PRT, -PIT], axis=0)    # [128, 128] (K-stacked)

    # direct fr path: fr_i = Rx @ f_i @ Cy^T (pure 2x Fourier upsampling)
    ExRm = np.cos(2 * np.pi * np.outer(kx, x) / 64)
    ExIm = -np.sin(2 * np.pi * np.outer(kx, x) / 64)
    EyRm = np.cos(2 * np.pi * np.outer(c, y) / 64)
    EyIm = -np.sin(2 * np.pi * np.outer(c, y) / 64)
    QRm = s_q * w_c[None, :] * np.cos(2 * np.pi * np.outer(Y, c) / 128)
    QIm = s_q * w_c[None, :] * np.sin(2 * np.pi * np.outer(Y, c) / 128)
    Rx = PRT.T @ ExRm - PIT.T @ ExIm                 # [128, 64] (PRT.T == PR)
    Cy = QRm @ EyRm - QIm @ EyIm                     # [128, 64]
    RxT = Rx.T                                       # [x=64, X=128]
    CyT = np.concatenate([Cy.T, Cy.T], axis=0)       # [128, 128] doubled rows

    f32 = lambda a: np.ascontiguousarray(a, dtype=np.float32)
    return dict(ExF=f32(ExF), EyCT=f32(EyCT), EyST=f32(EyST),
                S_sel=f32(S_sel), tsg=f32(tsg), QF1=f32(QF1), QF2=f32(QF2),
                PRTPnIT=f32(PRTPnIT), RxT=f32(RxT), CyT=f32(CyT))


def _rot90_kernel(k):
    # z[..., i, j] = k[..., (-j) mod n, i]
    y = np.swapaxes(k, -2, -1)
    return np.concatenate([y[..., :1], y[..., :0:-1]], axis=-1)


def _symmetric_kernel(k):
    k1 = k
    k2 = _rot90_kernel(k1)
    k3 = _rot90_kernel(k2)
    k4 = _rot90_kernel(k3)
    k5 = np.swapaxes(k1, -2, -1)
    k6 = _rot90_kernel(k5)
    k7 = _rot90_kernel(k6)
    k8 = _rot90_kernel(k7)
    return (k1 + k2 + k3 + k4 + k5 + k6 + k7 + k8) / 8.0


def _prep_k_all(kernel_np):
    """kernel [1,8,16,64,64] -> k_all [128, 2048] conv-layout packed."""
    ksym = _symmetric_kernel(kernel_np.astype(np.float64))[0]   # [8,16,64,64]
    K = np.fft.rfft2(ksym).real                                  # [8,16,64,33]
    Kc = np.transpose(K[:, :, :, :32], (0, 1, 3, 2)).copy()      # [i,j,c,kx]
    Kc[:, :, :, 32] = 0.0                                        # kx nyquist
    k_all = np.zeros((128, 2048), dtype=np.float32)
    for i in range(8):
        h, im = i // 4, i % 4
        for j in range(16):
            k_all[im * 32:(im + 1) * 32, j * 128 + h * 64: j * 128 + h * 64 + 64] = Kc[i, j]
    return k_all


# ---------------------------------------------------------------------------
# device program
# ---------------------------------------------------------------------------

def _bcast(ap, n, axis_pos=1):
    """Insert a zero-step broadcast dim of size n into an AP (after partition dim)."""
    dims = list(ap.ap)
    dims.insert(axis_pos, [0, n])
    return AP(ap.tensor, ap.offset, dims)


def _view(ap, offset_elems, dims):
    """Raw AP view on the same tensor: explicit offset (elems) + [step, count] dims."""
    return AP(ap.tensor, ap.offset + offset_elems, dims)


def build_program(reps=1, ablate=(), cross_bf16=False, gps_subs=False,
                  gps_conv=False, phase_b=False, dma_split=0, gcopy_dve=False,
                  gps_prod8=0):
    """ablate: subset of {'cross','synth','conv','dma'} to skip (profiling)."""
    nc = bacc.Bacc("TRN2", target_bir_lowering=False)
    consts = _host_consts()
    BF16 = mybir.dt.bfloat16
    xdt = BF16 if cross_bf16 else F32

    f_in = nc.dram_tensor("f_in", [B_PER_CORE, C1, 64, 64], F32, kind="ExternalInput")
    k_in = nc.dram_tensor("k_all", [128, 2048], F32, kind="ExternalInput")
    # transposed output layout [b, X, ch, Y]; host returns .transpose(0,2,1,3) view
    out_sh = nc.dram_tensor("out_sh", [B_PER_CORE, 128, NCH_OUT, 128], F32,
                            kind="ExternalOutput")

    cdr = {name: nc.inline_tensor(arr, name=f"c_{name}") for name, arr in consts.items()}

    with tile.TileContext(nc) as tc:
        with (
            tc.tile_pool(name="cp", bufs=1) as cp,
            tc.tile_pool(name="fld", bufs=1) as fld,     # u_all/v_all/fr_all
            tc.tile_pool(name="wk", bufs=2) as wk,       # small working tiles
            tc.tile_pool(name="mw", bufs=1) as mwp,      # conv wide tiles
            tc.tile_pool(name="wp", bufs=2) as wp,       # cross product blocks
            tc.tile_pool(name="crp", bufs=3) as crp,     # cross output staging
            tc.tile_pool(name="pp", bufs=1, space="PSUM") as pp,
        ):
            # ---- load constants ----
            cs = {}
            for name, arr in consts.items():
                t = cp.tile(list(arr.shape), F32, tag=f"c_{name}", name=f"cs_{name}")
                nc.sync.dma_start(out=t[:], in_=cdr[name][:])
                cs[name] = t
            k_sb = cp.tile([128, 2048], F32, tag="k_sb")
            nc.sync.dma_start(out=k_sb[:], in_=k_in[:])

            u_all = fld.tile([128, 16 * 256], xdt, tag="u_all")
            v_all = fld.tile([128, 16 * 256], xdt, tag="v_all")
            fr_all = fld.tile([128, 8 * 256], F32, tag="fr_all")

            dma_tick = [0]

            def out_dma(out_ap, in_ap, ring=None):
                # dma_split = modulus M: every M-th output DMA issues on the ACT ring
                if ring is not None:
                    eng = ring
                elif dma_split and dma_tick[0] % dma_split == dma_split - 1:
                    eng = nc.scalar
                else:
                    eng = nc.sync
                dma_tick[0] += 1
                eng.dma_start(out=out_ap, in_=in_ap)

            prod_tick = [0]

            def prod_eng():
                i = prod_tick[0] % 8
                prod_tick[0] += 1
                return nc.gpsimd if i < gps_prod8 else nc.vector

            def emit_cross_block(gI, gJ, b):
                """cross products for channel groups gI x gJ, one sample."""
                # late blocks drain on the otherwise-idle ACT ring
                ring = nc.scalar if (b == B_PER_CORE - 1 and gJ >= 2) else None
                W1 = wp.tile([128, 2048], xdt, tag="W1", name="W1")
                for ai in range(4):
                    a = 4 * gI + ai
                    in0 = _view(u_all[:], a * 256 + b * 128,
                                [u_all[:].ap[0], [0, 4], [1, 128]])
                    in1 = _view(v_all[:], gJ * 1024 + b * 128,
                                [v_all[:].ap[0], [256, 4], [1, 128]])
                    out = W1[:, ai * 512:(ai + 1) * 512].rearrange(
                        "p (cb f) -> p cb f", cb=4)
                    prod_eng().tensor_mul(out, in0, in1)
                if gI != gJ:
                    W2 = wp.tile([128, 2048], xdt, tag="W2", name="W2")
                    for bjl in range(4):
                        bj = 4 * gJ + bjl
                        in0 = _view(u_all[:], bj * 256 + b * 128,
                                    [u_all[:].ap[0], [0, 4], [1, 128]])
                        in1 = _view(v_all[:], gI * 1024 + b * 128,
                                    [v_all[:].ap[0], [256, 4], [1, 128]])
                        out = W2[:, bjl * 512:(bjl + 1) * 512].rearrange(
                            "p (ca f) -> p ca f", ca=4)
                        prod_eng().tensor_mul(out, in0, in1)
                    for ai in range(4):
                        a = 4 * gI + ai
                        cr = crp.tile([128, 512], F32, tag="cr", name="cr")
                        in0 = W1[:, ai * 512:(ai + 1) * 512].rearrange(
                            "p (cb f) -> p cb f", cb=4)
                        in1 = _view(W2[:], ai * 128,
                                    [W2[:].ap[0], [512, 4], [1, 128]])
                        sub_eng = nc.gpsimd if gps_subs else nc.vector
                        sub_eng.tensor_sub(
                            cr[:].rearrange("p (cb f) -> p cb f", cb=4), in0, in1)
                        pch = 8 + _PAIR_IDX[(a, 4 * gJ)]
                        if 'dma' not in ablate:
                            out_dma(out_sh[b, :, pch:pch + 4, :],
                                    cr[:].rearrange("x (c y) -> x c y", c=4), ring=ring)
                else:
                    for ai in range(3):
                        a = 4 * gI + ai
                        cnt = 3 - ai
                        cr = crp.tile([128, 512], F32, tag="cr", name="cr")
                        in0 = _view(W1[:], ai * 512 + (ai + 1) * 128,
                                    [W1[:].ap[0], [128, cnt], [1, 128]])
                        in1 = _view(W1[:], (ai + 1) * 512 + ai * 128,
                                    [W1[:].ap[0], [512, cnt], [1, 128]])
                        sub_eng = nc.gpsimd if gps_subs else nc.vector
                        sub_eng.tensor_sub(
                            cr[:, 0:cnt * 128].rearrange(
                                "p (cb f) -> p cb f", cb=cnt), in0, in1)
                        pch = 8 + _PAIR_IDX[(a, a + 1)]
                        if 'dma' not in ablate:
                            out_dma(out_sh[b, :, pch:pch + cnt, :],
                                    cr[:, 0:cnt * 128].rearrange("x (c y) -> x c y", c=cnt),
                                    ring=ring)

            def emit_stage1(b, st):
                A_ch = []
                T1s = []
                for ip in range(4):
                    fsb = wk.tile([64, 128], F32, tag="fsb", name="fsb")
                    nc.sync.dma_start(
                        out=fsb[:].rearrange("x (i y) -> x i y", i=2),
                        in_=f_in[b, 2 * ip:2 * ip + 2].rearrange("i x y -> x i y"))
                    psA = pp.tile([128, 192], F32, tag="bankA", bufs=2, name="psA")
                    nc.tensor.matmul(psA[:], fsb[:], cs["ExF"][:], start=True, stop=True)
                    for iloc in range(2):
                        a_t = wk.tile([64, 192], F32, tag=f"ach{2*ip+iloc}",
                                      name=f"ach{2*ip+iloc}")
                        nc.vector.tensor_copy(a_t[:], psA[iloc * 64:(iloc + 1) * 64, :])
                        A_ch.append(a_t)
                    # fr path: T1 = [f_i^T Rx^T | f_{i+1}^T Rx^T]  ([y, X] per channel)
                    psT1 = pp.tile([128, 128], F32, tag="bankA", bufs=2, name="psT1")
                    nc.tensor.matmul(psT1[:], fsb[:], cs["RxT"][:], start=True, stop=True)
                    t1sb = wk.tile([128, 128], F32, tag=f"t1sb{ip}", name=f"t1sb{ip}")
                    nc.scalar.copy(out=t1sb[:], in_=psT1[:])
                    T1s.append(t1sb)
                st['A_ch'] = A_ch
                st['T1s'] = T1s

            def emit_stage2(b, st):
                A_ch = st['A_ch']
                # out free = [F_R(kx64) | F_I(kx64)] per tile
                psFcv = [pp.tile([128, 128], F32, tag=f"bankF{4+h}", name=f"psFcv{h}")
                         for h in range(2)]
                EyC, EyS = cs["EyCT"], cs["EyST"]
                for i in range(8):
                    A_RI = A_ch[i][:, 0:128]     # [A_R | A_I]
                    A_IS = A_ch[i][:, 64:192]    # [A_I | -A_R]
                    h, im = i // 4, i % 4
                    sl = slice(im * 32, (im + 1) * 32)
                    tp = (0, im * 32)
                    nc.tensor.matmul(psFcv[h][sl, :], EyC[:], A_RI, start=True, stop=False,
                                     tile_position=tp)
                    nc.tensor.matmul(psFcv[h][sl, :], EyS[:], A_IS, start=False, stop=True,
                                     tile_position=tp)

                Fcv = wk.tile([128, 256], F32, tag="Fcv", name="Fcv")
                for h in range(2):
                    nc.vector.tensor_copy(Fcv[:, h * 64:(h + 1) * 64], psFcv[h][:, 0:64])
                    nc.vector.tensor_copy(Fcv[:, 128 + h * 64:128 + (h + 1) * 64],
                                          psFcv[h][:, 64:128])
                st['Fcv'] = Fcv

            def emit_conv(b, st):
                Fcv = st['Fcv']
                Mw = []
                for RI in range(2):
                    m_t = mwp.tile([128, 2048], F32, tag=f"mw{RI}", name=f"mw{RI}")
                    in0 = _bcast(Fcv[:, RI * 128:(RI + 1) * 128], 16)
                    conv_eng = nc.gpsimd if gps_conv else nc.vector
                    conv_eng.tensor_mul(
                        m_t[:].rearrange("p (j f) -> p j f", j=16),
                        in0,
                        k_sb[:].rearrange("p (j f) -> p j f", j=16))
                    Mw.append(m_t)

                BuR = wk.tile([32, 1024], F32, tag="BuR", name="BuR")
                BuI = wk.tile([32, 1024], F32, tag="BuI", name="BuI")
                BvR = wk.tile([32, 1024], F32, tag="BvR", name="BvR")
                BvI = wk.tile([32, 1024], F32, tag="BvI", name="BvI")
                tsg = cs["tsg"]
                for RI in range(2):
                    for jh in range(2):
                        ps_acv = pp.tile([32, 512], F32, tag="bankA", bufs=2, name="ps_acv")
                        for h in range(2):
                            rhs = _view(Mw[RI][:], jh * 1024 + h * 64,
                                        [Mw[RI][:].ap[0], [128, 8], [1, 64]])
                            nc.tensor.matmul(ps_acv[:], cs["S_sel"][:], rhs,
                                             start=(h == 0), stop=(h == 1))
                        osl = slice(jh * 512, (jh + 1) * 512)
                        if RI == 0:  # A_R -> imaginary parts of Bu/Bv
                            nc.vector.tensor_mul(BuI[:, osl], ps_acv[:], tsg[:, 512:1024])
                            nc.vector.tensor_mul(BvI[:, osl], ps_acv[:], tsg[:, 1536:2048])
                        else:        # A_I -> real parts (negated multipliers)
                            nc.vector.tensor_mul(BuR[:, osl], ps_acv[:], tsg[:, 0:512])
                            nc.vector.tensor_mul(BvR[:, osl], ps_acv[:], tsg[:, 1024:1536])
                st['B'] = (BuR, BuI, BvR, BvI)

            def emit_synth(b, st):
                BuR, BuI, BvR, BvI = st['B']

                # fr direct: fr_i = (T1_i)^T @ Cy^T via one matmul per channel
                for i in range(8):
                    ip, iloc = i // 2, i % 2
                    t1 = st['T1s'][ip][iloc * 64:(iloc + 1) * 64, :]
                    psUf = pp.tile([128, 128], F32, tag=f"bankF{2 + i % 2}", name="psUf")
                    nc.tensor.matmul(psUf[:], t1,
                                     cs["CyT"][iloc * 64:(iloc + 1) * 64, :],
                                     start=True, stop=True)
                    nc.scalar.copy(out=fr_all[:, i * 256 + b * 128:i * 256 + (b + 1) * 128],
                                   in_=psUf[:])
                if 'dma' not in ablate:
                    frv = _view(fr_all[:], b * 128,
                                [fr_all[:].ap[0], [256, 8], [1, 128]])
                    out_dma(out_sh[b, :, 0:8, :], frv)

                def bu_slices(cpair):
                    csl = slice(cpair * 128, (cpair + 1) * 128)
                    return (BuR[:, csl], BuI[:, csl])

                def bv_slices(cpair):
                    csl = slice(cpair * 128, (cpair + 1) * 128)
                    return (BvR[:, csl], BvI[:, csl])

                fields = [
                    (bu_slices, u_all, 16, False),
                    (bv_slices, v_all, 16, False),
                ]
                for get_sl, dest, nch, is_fr in fields:
                    for cpair in range(nch // 2):
                        BRs, BIs = get_sl(cpair)
                        psG = pp.tile([128, 256], F32, tag=f"bankF{cpair % 2}", name="psG")
                        nc.tensor.matmul(psG[:], BRs, cs["QF1"][:], start=True, stop=False)
                        nc.tensor.matmul(psG[:], BIs, cs["QF2"][:], start=False, stop=True)
                        # stacked [G_R ; G_I] x 2 channels -> one K=128 N=256 matmul
                        G_stk = wk.tile([128, 256], F32, tag="G_stk", name="G_stk")
                        for cl in range(2):
                            if gcopy_dve and cl == 1:
                                nc.vector.tensor_copy(G_stk[0:64, cl * 128:(cl + 1) * 128],
                                                      psG[cl * 64:(cl + 1) * 64, 0:128])
                                nc.vector.tensor_copy(G_stk[64:128, cl * 128:(cl + 1) * 128],
                                                      psG[cl * 64:(cl + 1) * 64, 128:256])
                            else:
                                nc.scalar.copy(out=G_stk[0:64, cl * 128:(cl + 1) * 128],
                                               in_=psG[cl * 64:(cl + 1) * 64, 0:128])
                                nc.scalar.copy(out=G_stk[64:128, cl * 128:(cl + 1) * 128],
                                               in_=psG[cl * 64:(cl + 1) * 64, 128:256])
                        psU = pp.tile([128, 256], F32, tag=f"bankF{2 + cpair % 2}", name="psU")
                        nc.tensor.matmul(psU[:], cs["PRTPnIT"][:], G_stk[:],
                                         start=True, stop=True)
                        dsl = _view(dest[:], (2 * cpair) * 256 + b * 128,
                                    [dest[:].ap[0], [256, 2], [1, 128]])
                        if is_fr:
                            nc.scalar.copy(out=dsl, in_=psU[:].rearrange(
                                "p (c y) -> p c y", c=2))
                        else:
                            nc.vector.tensor_copy(dsl, psU[:].rearrange(
                                "p (c y) -> p c y", c=2))


            def emit_cross(b, st):
                for gI in range(4):
                    for gJ in range(gI, 4):
                        emit_cross_block(gI, gJ, b)

            for rep in range(reps):
                st = {b: {} for b in range(B_PER_CORE)}
                for b in range(B_PER_CORE):
                    emit_stage1(b, st[b])
                for b in range(B_PER_CORE):
                    emit_stage2(b, st[b])
                if 'conv' in ablate:
                    continue
                for b in range(B_PER_CORE):
                    emit_conv(b, st[b])
                if 'synth' in ablate:
                    continue
                for b in range(B_PER_CORE):
                    emit_synth(b, st[b])
                if 'cross' in ablate:
                    continue
                for b in range(B_PER_CORE):
                    emit_cross(b, st[b])
    nc.compile()
    return nc


# ---------------------------------------------------------------------------
# entry point
# ---------------------------------------------------------------------------

_PROGRAM = {}


def _get_program(reps=1, ablate=(), cross_bf16=None, **kw):
    global _PROGRAM
    import os
    if cross_bf16 is None:
        cross_bf16 = bool(os.environ.get("KBF16"))
    if 'gps_subs' not in kw:
        kw['gps_subs'] = os.environ.get("KGPS", "1") == "1"
    if 'gps_prod8' not in kw:
        kw['gps_prod8'] = int(os.environ.get("KGPSP", "4"))
    if 'gps_conv' not in kw:
        kw['gps_conv'] = os.environ.get("KGPSC", "1") == "1"
    if 'phase_b' not in kw and os.environ.get("KPHB"):
        kw['phase_b'] = True
    key = (reps, tuple(sorted(ablate)), cross_bf16, tuple(sorted(kw.items())))
    if key not in _PROGRAM:
        _PROGRAM[key] = build_program(reps, ablate=ablate, cross_bf16=cross_bf16, **kw)
    return _PROGRAM[key]


LAST_EXEC_NS = None
LAST_RESULT = None


def kernel(f, kernel):
    global LAST_EXEC_NS, LAST_RESULT
    f = np.ascontiguousarray(f, dtype=np.float32)
    k_all = _prep_k_all(np.asarray(kernel))
    nc = _get_program()
    in_maps = [
        {"f_in": f[2 * c:2 * c + 2], "k_all": k_all} for c in range(N_CORES)
    ]
    import os
    trace = bool(os.environ.get("KERNEL_TRACE"))
    res = run_bass_kernel_spmd(nc, in_maps, list(range(N_CORES)), trace=trace)
    LAST_RESULT = res
    if res.exec_time_ns is not None:
        LAST_EXEC_NS = res.exec_time_ns
    out = np.concatenate([res.results[c]["out_sh"] for c in range(N_CORES)], axis=0)
    # device layout is [b, X, ch, Y]; return the [b, ch, X, Y] view
    return out.transpose(0, 2, 1, 3)



# revision 11
# speedup vs baseline: 1.1947x; 1.1947x over previous
"""Trainium2 Bass kernel for nn_EquivariantLayer (spectral equivariant layer).

Strategy (data-parallel over batch, 2 samples/core x 8 cores):
  All FFTs are expressed as real DFT matmuls on the TensorEngine with layouts
  chosen so no corner-turn transposes are ever needed:

    stage1:  A = f^T @ [ExR^T | ExI^T]          (contract x; out [y, (RI,kx)])
    stage2:  F = Ey @ A                          (contract y; out [c, kx], c-major)
             -> two layouts: conv layout [(i%4)*32+c, kx] and fr layout [c, (i,kx)]
    conv:    M = F (*) K elementwise (K = rfft2(sym kernel) is REAL since the
             symmetrized kernel is D4-symmetric); i-reduction via a selector
             matmul on the TensorEngine (PSUM accumulation over i-halves)
    uncurl:  TO_U = i*t, TO_V = i*s are pure-imaginary -> 2 real mults each
    synth:   field = Re(P @ B^T_cm @ Q^T) as two matmul stages (stage a/b)
    cross:   u_a v_b - u_b v_a on the VectorEngine with zero-step broadcast APs

Output [16, 128, 128, 128] f32 (~134 MB) dominates traffic (memory regime).
"""
import sys
import numpy as np

if '/opt/trn_rl_repo' not in sys.path:
    sys.path.insert(0, '/opt/trn_rl_repo')

import concourse.bass as bass
from concourse import bacc
import concourse.mybir as mybir
import concourse.tile as tile
from concourse.bass import AP
from concourse.bass_utils import run_bass_kernel_spmd

F32 = mybir.dt.float32
N_CORES = 8
B_PER_CORE = 2
C1, C2, N1, N2 = 8, 16, 64, 128
NCH_OUT = 128  # 8 fr + 120 cross

I_IDX, J_IDX = np.triu_indices(C2, 1)
_PAIR_IDX = {}
for _p, (_a, _b) in enumerate(zip(I_IDX, J_IDX)):
    _PAIR_IDX[(int(_a), int(_b))] = _p


# ---------------------------------------------------------------------------
# host-side constant construction
# ---------------------------------------------------------------------------

def _host_consts():
    x = np.arange(64)
    kx = np.arange(64)
    c = np.arange(32)
    y = np.arange(64)
    X = np.arange(128)
    Y = np.arange(128)

    FRs = np.where(kx <= 32, kx, kx - 64).astype(np.float64)  # signed row freq

    ExR = np.cos(2 * np.pi * np.outer(kx, x) / 64)   # [kx, x]
    ExI = -np.sin(2 * np.pi * np.outer(kx, x) / 64)
    # [A_R | A_I | -A_R] so stage2 fuses R/I into two matmuls
    ExF = np.concatenate([ExR.T, ExI.T, -ExR.T], axis=1)   # [x, 192]

    # F_R = C A_R + S A_I ; F_I = C A_I + S (-A_R)   (C=cos, S=sin)
    EyCT = np.cos(2 * np.pi * np.outer(c, y) / 64).T   # [y=64, c=32]
    EyST = np.sin(2 * np.pi * np.outer(c, y) / 64).T

    S_sel = np.zeros((128, 32))
    for im in range(4):
        S_sel[im * 32 + np.arange(32), np.arange(32)] = 1.0

    den = FRs[None, :] ** 2 + c[:, None].astype(np.float64) ** 2
    den[0, 0] = 1.0
    t_u = c[:, None] / den                           # [32, 64]
    s_v = -FRs[None, :] / den
    t_rep = np.tile(t_u, (1, 8))                     # [32, 512] (j-rep)
    s_rep = np.tile(s_v, (1, 8))
    tsg = np.concatenate([-t_rep, t_rep, -s_rep, s_rep], axis=1)  # [32, 2048]

    w_c = np.where(c == 0, 1.0, 2.0)
    s_q = 2.0 / (128.0 * 128.0)
    QRT = (s_q * w_c[None, :] * np.cos(2 * np.pi * np.outer(Y, c) / 128)).T  # [c, Y]
    QIT = (s_q * w_c[None, :] * np.sin(2 * np.pi * np.outer(Y, c) / 128)).T
    QF1 = np.concatenate([QRT, QIT], axis=1)         # [32, 256]
    QF2 = np.concatenate([-QIT, QRT], axis=1)

    PRT = np.cos(2 * np.pi * np.outer(FRs, X) / 128)   # [r=64, X=128]
    PIT = np.sin(2 * np.pi * np.outer(FRs, X) / 128)
    PRT[32, :] = 0.0
    PIT[32, :] = 0.0
    PRTPnIT = np.concatenate([PRT, -PIT], axis=0)    # [128, 128] (K-stacked)

    # direct fr path: fr_i = Rx @ f_i @ Cy^T (pure 2x Fourier upsampling)
    ExRm = np.cos(2 * np.pi * np.outer(kx, x) / 64)
    ExIm = -np.sin(2 * np.pi * np.outer(kx, x) / 64)
    EyRm = np.cos(2 * np.pi * np.outer(c, y) / 64)
    EyIm = -np.sin(2 * np.pi * np.outer(c, y) / 64)
    QRm = s_q * w_c[None, :] * np.cos(2 * np.pi * np.outer(Y, c) / 128)
    QIm = s_q * w_c[None, :] * np.sin(2 * np.pi * np.outer(Y, c) / 128)
    Rx = PRT.T @ ExRm - PIT.T @ ExIm                 # [128, 64] (PRT.T == PR)
    Cy = QRm @ EyRm - QIm @ EyIm                     # [128, 64]
    RxT = Rx.T                                       # [x=64, X=128]
    CyT = np.concatenate([Cy.T, Cy.T], axis=0)       # [128, 128] doubled rows

    # fused stage1 moving operand: [A_R | A_I | -A_R | fr-T1] in one matmul
    ExFR = np.concatenate([ExF, RxT], axis=1)        # [x, 320]

    f32 = lambda a: np.ascontiguousarray(a, dtype=np.float32)
    return dict(ExFR=f32(ExFR), EyCT=f32(EyCT), EyST=f32(EyST),
                S_sel=f32(S_sel), tsg=f32(tsg), QF1=f32(QF1), QF2=f32(QF2),
                PRTPnIT=f32(PRTPnIT), CyT=f32(CyT))


def _rot90_kernel(k):
    # z[..., i, j] = k[..., (-j) mod n, i]
    y = np.swapaxes(k, -2, -1)
    return np.concatenate([y[..., :1], y[..., :0:-1]], axis=-1)


def _symmetric_kernel(k):
    k1 = k
    k2 = _rot90_kernel(k1)
    k3 = _rot90_kernel(k2)
    k4 = _rot90_kernel(k3)
    k5 = np.swapaxes(k1, -2, -1)
    k6 = _rot90_kernel(k5)
    k7 = _rot90_kernel(k6)
    k8 = _rot90_kernel(k7)
    return (k1 + k2 + k3 + k4 + k5 + k6 + k7 + k8) / 8.0


def _prep_k_all(kernel_np):
    """kernel [1,8,16,64,64] -> k_all [128, 2048] conv-layout packed."""
    ksym = _symmetric_kernel(kernel_np.astype(np.float64))[0]   # [8,16,64,64]
    K = np.fft.rfft2(ksym).real                                  # [8,16,64,33]
    Kc = np.transpose(K[:, :, :, :32], (0, 1, 3, 2)).copy()      # [i,j,c,kx]
    Kc[:, :, :, 32] = 0.0                                        # kx nyquist
    k_all = np.zeros((128, 2048), dtype=np.float32)
    for i in range(8):
        h, im = i // 4, i % 4
        for j in range(16):
            k_all[im * 32:(im + 1) * 32, j * 128 + h * 64: j * 128 + h * 64 + 64] = Kc[i, j]
    return k_all


# ---------------------------------------------------------------------------
# device program
# ---------------------------------------------------------------------------

def _r(ap):
    """Bitcast an fp32 AP to float32r (1 cycle/row PE mode when N>=256)."""
    return ap.bitcast(mybir.dt.float32r)


def _bcast(ap, n, axis_pos=1):
    """Insert a zero-step broadcast dim of size n into an AP (after partition dim)."""
    dims = list(ap.ap)
    dims.insert(axis_pos, [0, n])
    return AP(ap.tensor, ap.offset, dims)


def _view(ap, offset_elems, dims):
    """Raw AP view on the same tensor: explicit offset (elems) + [step, count] dims."""
    return AP(ap.tensor, ap.offset + offset_elems, dims)


def build_program(reps=1, ablate=(), cross_bf16=False, gps_subs=False,
                  gps_conv=False, phase_b=False, dma_split=0, gcopy_dve=False,
                  gps_prod8=0):
    """ablate: subset of {'cross','synth','conv','dma'} to skip (profiling)."""
    nc = bacc.Bacc("TRN2", target_bir_lowering=False)
    consts = _host_consts()
    BF16 = mybir.dt.bfloat16
    xdt = BF16 if cross_bf16 else F32

    f_in = nc.dram_tensor("f_in", [B_PER_CORE, C1, 64, 64], F32, kind="ExternalInput")
    k_in = nc.dram_tensor("k_all", [128, 2048], F32, kind="ExternalInput")
    # transposed output layout [b, X, ch, Y]; host returns .transpose(0,2,1,3) view
    out_sh = nc.dram_tensor("out_sh", [B_PER_CORE, 128, NCH_OUT, 128], F32,
                            kind="ExternalOutput")

    cdr = {name: nc.inline_tensor(arr, name=f"c_{name}") for name, arr in consts.items()}

    with tile.TileContext(nc) as tc:
        with (
            tc.tile_pool(name="cp", bufs=1) as cp,
            tc.tile_pool(name="fld", bufs=1) as fld,     # u_all/v_all/fr_all
            tc.tile_pool(name="wk", bufs=2) as wk,       # small working tiles
            tc.tile_pool(name="mw", bufs=1) as mwp,      # conv wide tiles
            tc.tile_pool(name="wp", bufs=2) as wp,       # cross product blocks
            tc.tile_pool(name="crp", bufs=3) as crp,     # cross output staging
            tc.tile_pool(name="pp", bufs=1, space="PSUM") as pp,
        ):
            # ---- load constants (spread across rings; stage1's ExFR first on sync) ----
            const_ring = {"ExFR": nc.sync, "EyCT": nc.gpsimd, "EyST": nc.gpsimd,
                          "S_sel": nc.gpsimd, "tsg": nc.scalar, "QF1": nc.scalar,
                          "QF2": nc.scalar, "PRTPnIT": nc.scalar, "CyT": nc.scalar}
            cs = {}
            for name, arr in consts.items():
                t = cp.tile(list(arr.shape), F32, tag=f"c_{name}", name=f"cs_{name}")
                const_ring[name].dma_start(out=t[:], in_=cdr[name][:])
                cs[name] = t
            k_sb = cp.tile([128, 2048], F32, tag="k_sb")
            nc.gpsimd.dma_start(out=k_sb[:], in_=k_in[:])

            u_all = fld.tile([128, 16 * 256], xdt, tag="u_all")
            v_all = fld.tile([128, 16 * 256], xdt, tag="v_all")
            fr_all = fld.tile([128, 8 * 256], F32, tag="fr_all")

            dma_tick = [0]

            def out_dma(out_ap, in_ap, ring=None):
                # dma_split = modulus M: every M-th output DMA issues on the ACT ring
                if ring is not None:
                    eng = ring
                elif dma_split and dma_tick[0] % dma_split == dma_split - 1:
                    eng = nc.scalar
                else:
                    eng = nc.sync
                dma_tick[0] += 1
                eng.dma_start(out=out_ap, in_=in_ap)

            prod_tick = [0]

            def prod_eng():
                i = prod_tick[0] % 8
                prod_tick[0] += 1
                return nc.gpsimd if i < gps_prod8 else nc.vector

            def emit_cross_block(gI, gJ, b):
                """cross products for channel groups gI x gJ, one sample."""
                # late blocks drain on the otherwise-idle ACT ring
                ring = nc.scalar if (b == B_PER_CORE - 1 and gJ >= 2) else None
                W1 = wp.tile([128, 2048], xdt, tag="W1", name="W1")
                for ai in range(4):
                    a = 4 * gI + ai
                    in0 = _view(u_all[:], a * 256 + b * 128,
                                [u_all[:].ap[0], [0, 4], [1, 128]])
                    in1 = _view(v_all[:], gJ * 1024 + b * 128,
                                [v_all[:].ap[0], [256, 4], [1, 128]])
                    out = W1[:, ai * 512:(ai + 1) * 512].rearrange(
                        "p (cb f) -> p cb f", cb=4)
                    prod_eng().tensor_mul(out, in0, in1)
                if gI != gJ:
                    W2 = wp.tile([128, 2048], xdt, tag="W2", name="W2")
                    for bjl in range(4):
                        bj = 4 * gJ + bjl
                        in0 = _view(u_all[:], bj * 256 + b * 128,
                                    [u_all[:].ap[0], [0, 4], [1, 128]])
                        in1 = _view(v_all[:], gI * 1024 + b * 128,
                                    [v_all[:].ap[0], [256, 4], [1, 128]])
                        out = W2[:, bjl * 512:(bjl + 1) * 512].rearrange(
                            "p (ca f) -> p ca f", ca=4)
                        prod_eng().tensor_mul(out, in0, in1)
                    for ai in range(4):
                        a = 4 * gI + ai
                        cr = crp.tile([128, 512], F32, tag="cr", name="cr")
                        in0 = W1[:, ai * 512:(ai + 1) * 512].rearrange(
                            "p (cb f) -> p cb f", cb=4)
                        in1 = _view(W2[:], ai * 128,
                                    [W2[:].ap[0], [512, 4], [1, 128]])
                        sub_eng = nc.gpsimd if gps_subs else nc.vector
                        sub_eng.tensor_sub(
                            cr[:].rearrange("p (cb f) -> p cb f", cb=4), in0, in1)
                        pch = 8 + _PAIR_IDX[(a, 4 * gJ)]
                        if 'dma' not in ablate:
                            out_dma(out_sh[b, :, pch:pch + 4, :],
                                    cr[:].rearrange("x (c y) -> x c y", c=4), ring=ring)
                else:
                    for ai in range(3):
                        a = 4 * gI + ai
                        cnt = 3 - ai
                        cr = crp.tile([128, 512], F32, tag="cr", name="cr")
                        in0 = _view(W1[:], ai * 512 + (ai + 1) * 128,
                                    [W1[:].ap[0], [128, cnt], [1, 128]])
                        in1 = _view(W1[:], (ai + 1) * 512 + ai * 128,
                                    [W1[:].ap[0], [512, cnt], [1, 128]])
                        sub_eng = nc.gpsimd if gps_subs else nc.vector
                        sub_eng.tensor_sub(
                            cr[:, 0:cnt * 128].rearrange(
                                "p (cb f) -> p cb f", cb=cnt), in0, in1)
                        pch = 8 + _PAIR_IDX[(a, a + 1)]
                        if 'dma' not in ablate:
                            out_dma(out_sh[b, :, pch:pch + cnt, :],
                                    cr[:, 0:cnt * 128].rearrange("x (c y) -> x c y", c=cnt),
                                    ring=ring)

            def emit_stage1(b, st):
                A_ch = []
                T1s = []
                for ip in range(4):
                    fsb = wk.tile([64, 128], F32, tag="fsb", name="fsb")
                    nc.sync.dma_start(
                        out=fsb[:].rearrange("x (i y) -> x i y", i=2),
                        in_=f_in[b, 2 * ip:2 * ip + 2].rearrange("i x y -> x i y"))
                    # fused: [A_R | A_I | -A_R | T1] in one N=320 fp32r matmul
                    psA = pp.tile([128, 320], F32, tag="bankA", bufs=2, name="psA")
                    nc.tensor.matmul(psA[:], _r(fsb[:]), _r(cs["ExFR"][:]),
                                     start=True, stop=True)
                    for iloc in range(2):
                        a_t = wk.tile([64, 192], F32, tag=f"ach{2*ip+iloc}",
                                      name=f"ach{2*ip+iloc}")
                        nc.vector.tensor_copy(a_t[:], psA[iloc * 64:(iloc + 1) * 64, 0:192])
                        A_ch.append(a_t)
                    t1sb = wk.tile([128, 128], F32, tag=f"t1sb{ip}", name=f"t1sb{ip}")
                    nc.scalar.copy(out=t1sb[:], in_=psA[:, 192:320])
                    T1s.append(t1sb)
                st['A_ch'] = A_ch
                st['T1s'] = T1s

            def emit_stage2(b, st):
                A_ch = st['A_ch']
                # out free = [F_R(kx64) | F_I(kx64)] per tile
                psFcv = [pp.tile([128, 128], F32, tag=f"bankF{4+h}", name=f"psFcv{h}")
                         for h in range(2)]
                EyC, EyS = cs["EyCT"], cs["EyST"]
                for i in range(8):
                    A_RI = A_ch[i][:, 0:128]     # [A_R | A_I]
                    A_IS = A_ch[i][:, 64:192]    # [A_I | -A_R]
                    h, im = i // 4, i % 4
                    sl = slice(im * 32, (im + 1) * 32)
                    tp = (0, im * 32)
                    nc.tensor.matmul(psFcv[h][sl, :], _r(EyC[:]), _r(A_RI),
                                     start=True, stop=False, tile_position=tp)
                    nc.tensor.matmul(psFcv[h][sl, :], _r(EyS[:]), _r(A_IS),
                                     start=False, stop=True, tile_position=tp)

                Fcv = wk.tile([128, 256], F32, tag="Fcv", name="Fcv")
                for h in range(2):
                    nc.vector.tensor_copy(Fcv[:, h * 64:(h + 1) * 64], psFcv[h][:, 0:64])
                    nc.vector.tensor_copy(Fcv[:, 128 + h * 64:128 + (h + 1) * 64],
                                          psFcv[h][:, 64:128])
                st['Fcv'] = Fcv

            def emit_conv(b, st):
                Fcv = st['Fcv']
                Mw = []
                for RI in range(2):
                    m_t = mwp.tile([128, 2048], F32, tag=f"mw{RI}", name=f"mw{RI}")
                    in0 = _bcast(Fcv[:, RI * 128:(RI + 1) * 128], 16)
                    conv_eng = nc.gpsimd if gps_conv else nc.vector
                    conv_eng.tensor_mul(
                        m_t[:].rearrange("p (j f) -> p j f", j=16),
                        in0,
                        k_sb[:].rearrange("p (j f) -> p j f", j=16))
                    Mw.append(m_t)

                BuR = wk.tile([32, 1024], F32, tag="BuR", name="BuR")
                BuI = wk.tile([32, 1024], F32, tag="BuI", name="BuI")
                BvR = wk.tile([32, 1024], F32, tag="BvR", name="BvR")
                BvI = wk.tile([32, 1024], F32, tag="BvI", name="BvI")
                tsg = cs["tsg"]
                for RI in range(2):
                    for jh in range(2):
                        ps_acv = pp.tile([32, 512], F32, tag="bankA", bufs=2, name="ps_acv")
                        for h in range(2):
                            rhs = _view(Mw[RI][:], jh * 1024 + h * 64,
                                        [Mw[RI][:].ap[0], [128, 8], [1, 64]])
                            nc.tensor.matmul(ps_acv[:], _r(cs["S_sel"][:]), _r(rhs),
                                             start=(h == 0), stop=(h == 1))
                        osl = slice(jh * 512, (jh + 1) * 512)
                        if RI == 0:  # A_R -> imaginary parts of Bu/Bv
                            nc.vector.tensor_mul(BuI[:, osl], ps_acv[:], tsg[:, 512:1024])
                            nc.vector.tensor_mul(BvI[:, osl], ps_acv[:], tsg[:, 1536:2048])
                        else:        # A_I -> real parts (negated multipliers)
                            nc.vector.tensor_mul(BuR[:, osl], ps_acv[:], tsg[:, 0:512])
                            nc.vector.tensor_mul(BvR[:, osl], ps_acv[:], tsg[:, 1024:1536])
                st['B'] = (BuR, BuI, BvR, BvI)

            def emit_synth(b, st):
                BuR, BuI, BvR, BvI = st['B']

                # fr direct: fr_i = (T1_i)^T @ Cy^T via one matmul per channel
                for i in range(8):
                    ip, iloc = i // 2, i % 2
                    t1 = st['T1s'][ip][iloc * 64:(iloc + 1) * 64, :]
                    psUf = pp.tile([128, 128], F32, tag=f"bankF{2 + i % 2}", name="psUf")
                    nc.tensor.matmul(psUf[:], _r(t1),
                                     _r(cs["CyT"][iloc * 64:(iloc + 1) * 64, :]),
                                     start=True, stop=True)
                    nc.scalar.copy(out=fr_all[:, i * 256 + b * 128:i * 256 + (b + 1) * 128],
                                   in_=psUf[:])
                if 'dma' not in ablate:
                    frv = _view(fr_all[:], b * 128,
                                [fr_all[:].ap[0], [256, 8], [1, 128]])
                    out_dma(out_sh[b, :, 0:8, :], frv)

                def bu_slices(cpair):
                    csl = slice(cpair * 128, (cpair + 1) * 128)
                    return (BuR[:, csl], BuI[:, csl])

                def bv_slices(cpair):
                    csl = slice(cpair * 128, (cpair + 1) * 128)
                    return (BvR[:, csl], BvI[:, csl])

                fields = [
                    (bu_slices, u_all, 16, False),
                    (bv_slices, v_all, 16, False),
                ]
                for get_sl, dest, nch, is_fr in fields:
                    for cpair in range(nch // 2):
                        BRs, BIs = get_sl(cpair)
                        psG = pp.tile([128, 256], F32, tag=f"bankF{cpair % 2}", name="psG")
                        nc.tensor.matmul(psG[:], _r(BRs), _r(cs["QF1"][:]),
                                         start=True, stop=False)
                        nc.tensor.matmul(psG[:], _r(BIs), _r(cs["QF2"][:]),
                                         start=False, stop=True)
                        # stacked [G_R ; G_I] x 2 channels -> one K=128 N=256 matmul
                        G_stk = wk.tile([128, 256], F32, tag="G_stk", name="G_stk")
                        for cl in range(2):
                            if gcopy_dve and cl == 1:
                                nc.vector.tensor_copy(G_stk[0:64, cl * 128:(cl + 1) * 128],
                                                      psG[cl * 64:(cl + 1) * 64, 0:128])
                                nc.vector.tensor_copy(G_stk[64:128, cl * 128:(cl + 1) * 128],
                                                      psG[cl * 64:(cl + 1) * 64, 128:256])
                            else:
                                nc.scalar.copy(out=G_stk[0:64, cl * 128:(cl + 1) * 128],
                                               in_=psG[cl * 64:(cl + 1) * 64, 0:128])
                                nc.scalar.copy(out=G_stk[64:128, cl * 128:(cl + 1) * 128],
                                               in_=psG[cl * 64:(cl + 1) * 64, 128:256])
                        psU = pp.tile([128, 256], F32, tag=f"bankF{2 + cpair % 2}", name="psU")
                        nc.tensor.matmul(psU[:], _r(cs["PRTPnIT"][:]), _r(G_stk[:]),
                                         start=True, stop=True)
                        dsl = _view(dest[:], (2 * cpair) * 256 + b * 128,
                                    [dest[:].ap[0], [256, 2], [1, 128]])
                        if is_fr:
                            nc.scalar.copy(out=dsl, in_=psU[:].rearrange(
                                "p (c y) -> p c y", c=2))
                        else:
                            nc.vector.tensor_copy(dsl, psU[:].rearrange(
                                "p (c y) -> p c y", c=2))


            def emit_cross(b, st):
                for gI in range(4):
                    for gJ in range(gI, 4):
                        emit_cross_block(gI, gJ, b)

            for rep in range(reps):
                st = {b: {} for b in range(B_PER_CORE)}
                for b in range(B_PER_CORE):
                    emit_stage1(b, st[b])
                for b in range(B_PER_CORE):
                    emit_stage2(b, st[b])
                if 'conv' in ablate:
                    continue
                for b in range(B_PER_CORE):
                    emit_conv(b, st[b])
                if 'synth' in ablate:
                    continue
                for b in range(B_PER_CORE):
                    emit_synth(b, st[b])
                if 'cross' in ablate:
                    continue
                for b in range(B_PER_CORE):
                    emit_cross(b, st[b])
    nc.compile()
    return nc


# ---------------------------------------------------------------------------
# entry point
# ---------------------------------------------------------------------------

_PROGRAM = {}


def _get_program(reps=1, ablate=(), cross_bf16=None, **kw):
    global _PROGRAM
    import os
    if cross_bf16 is None:
        cross_bf16 = bool(os.environ.get("KBF16"))
    if 'gps_subs' not in kw:
        kw['gps_subs'] = os.environ.get("KGPS", "1") == "1"
    if 'gps_prod8' not in kw:
        kw['gps_prod8'] = int(os.environ.get("KGPSP", "4"))
    if 'gps_conv' not in kw:
        kw['gps_conv'] = os.environ.get("KGPSC", "1") == "1"
    if 'phase_b' not in kw and os.environ.get("KPHB"):
        kw['phase_b'] = True
    key = (reps, tuple(sorted(ablate)), cross_bf16, tuple(sorted(kw.items())))
    if key not in _PROGRAM:
        _PROGRAM[key] = build_program(reps, ablate=ablate, cross_bf16=cross_bf16, **kw)
    return _PROGRAM[key]


LAST_EXEC_NS = None
LAST_RESULT = None


def kernel(f, kernel):
    global LAST_EXEC_NS, LAST_RESULT
    f = np.ascontiguousarray(f, dtype=np.float32)
    k_all = _prep_k_all(np.asarray(kernel))
    nc = _get_program()
    in_maps = [
        {"f_in": f[2 * c:2 * c + 2], "k_all": k_all} for c in range(N_CORES)
    ]
    import os
    trace = bool(os.environ.get("KERNEL_TRACE"))
    res = run_bass_kernel_spmd(nc, in_maps, list(range(N_CORES)), trace=trace)
    LAST_RESULT = res
    if res.exec_time_ns is not None:
        LAST_EXEC_NS = res.exec_time_ns
    out = np.concatenate([res.results[c]["out_sh"] for c in range(N_CORES)], axis=0)
    # device layout is [b, X, ch, Y]; return the [b, ch, X, Y] view
    return out.transpose(0, 2, 1, 3)

